# revision 1
# baseline (speedup 1.0000x reference)
"""Trainium2 Bass kernel for the Dial2vec contrastive loss (nn_Dial2vec).

Math: the dense reference computes, per sequence,
    q = h * a[:,None]; r = h * b[:,None]               (a/b = role-0/1 masks)
    w = q @ r^T; fw = w * band                         (band from turn ids)
    q_cross = fw^T @ q; r_cross = fw @ r
then masked means of q / q_cross / r / r_cross, cosine similarities, and a
label-weighted log-softmax loss.

Because band[i,j] depends only on (turn_i, turn_j) and a*b == 0, everything
collapses to per-turn segment sums over the 16 turns:
    Q_T[t] = sum_{turn_l = t} a_l h_l;  R_T[t] likewise with b     [16, H]
    g_l    = a_l (Band @ R_T)[turn_l] + b_l (Band @ Q_T)[turn_l]   [L, H]
    gam_l  = h_l . g_l
    qs = sum a_l h_l; qc = sum a_l gam_l h_l; rs/rc likewise with b
and cosine similarity is scale-invariant, so the mask-count denominators
cancel (the 1e-8 norm clamps cannot trigger with this data).

The band smear (Band @ ...) is folded into host-precomputed 0/1 matrices:
with ABX[l, 0:16] = b_l * Band[turn_l, :], ABX[l, 16:32] = a_l * Band[turn_l, :],
    g = ABX @ [Q_T; R_T].

Device work per core (one dialogue = 10 sequences, data-parallel over 8
cores): three thin bf16 matmul stages on the PE plus, per 128-token chunk,
one DVE product (gam integrand) and one ACT copy-with-accumulate (the
row-sum). The host performs index-only preprocessing (one-hot / band-smeared
masks, bf16 casts) and the final O(B*H) cosine/log-softmax reduction over
the 40 gathered fp32 vectors per core.
"""

import os

import numpy as np

B_SEQ = 80
L = 512
H = 768
SAMPLES = 10
T = 16
VIEW_RANGE = 2
TEMP = 0.2
AVG_EPS = 1e-6
COS_EPS = 1e-8

N_CORES = 8
SPC = SAMPLES  # sequences per core = one dialogue
P = 128
LC = 384  # compacted token count (attention_mask=1 tokens only, zero-padded)
CHUNKS = LC // P  # 3
N_SPLITS = ((0, 512), (512, 768))  # PSUM-bank-aligned fp32 free-dim splits

_CACHE: dict = {}


def _build_nc(repeat: int = 1):
    """Build + compile the per-core Bass program (identical on all cores).

    repeat > 1 emits the whole program body N times (same tensors) — used
    only for wall-clock benchmarking of the steady-state iteration time.
    """
    from contextlib import ExitStack

    import concourse.bacc as bacc
    import concourse.mybir as mybir
    import concourse.tile as tile

    f32 = mybir.dt.float32
    bf16 = mybir.dt.bfloat16
    copy_fn = mybir.ActivationFunctionType.Copy

    nc = bacc.Bacc(
        "TRN2",
        debug=False,
        enable_asserts=False,
        target_bir_lowering=False,
    )

    # one row-block of 128 tokens per chunk; chunks side by side in free dim
    hid = nc.dram_tensor("hid", [SPC, P, CHUNKS * H], bf16, kind="ExternalInput").ap()
    ab = nc.dram_tensor("ab", [SPC, P, CHUNKS * 2 * T], bf16, kind="ExternalInput").ap()
    # pre-staged stage-D weights: per chunk the [a, b, 0, 0] columns
    # (chunk 0 zero-padded to 32 cols so stage D can init its psum strip);
    # the device fills the gam columns in place
    dc = nc.dram_tensor(
        "dc", [SPC, P, 2 * T + 4 * (CHUNKS - 1)], bf16, kind="ExternalInput"
    ).ap()
    abx = nc.dram_tensor("abx", [SPC, 2 * T, LC], bf16, kind="ExternalInput").ap()
    out = nc.dram_tensor("out", [4 * SPC, H], f32, kind="ExternalOutput").ap()

    # PE tile_position restricts psum output base partitions to {0,32,64},
    # so supergroups hold at most 3 sequences (32-row strips).
    GROUPS = [list(range(g, min(g + 3, SPC))) for g in range(0, SPC, 3)]

    with tile.TileContext(nc) as tc, ExitStack() as ctx:
        hp = ctx.enter_context(tc.tile_pool(name="hp", bufs=6))
        abp = ctx.enter_context(tc.tile_pool(name="abp", bufs=6))
        axp = ctx.enter_context(tc.tile_pool(name="axp", bufs=6))
        s1p = ctx.enter_context(tc.tile_pool(name="s1p", bufs=2))
        scp = ctx.enter_context(tc.tile_pool(name="scp", bufs=3))
        sc2p = ctx.enter_context(tc.tile_pool(name="sc2p", bufs=3))
        gmp = ctx.enter_context(tc.tile_pool(name="gmp", bufs=8))
        dcp = ctx.enter_context(tc.tile_pool(name="dcp", bufs=8))
        osp = ctx.enter_context(tc.tile_pool(name="osp", bufs=2))
        # one shared psum pool: every tile is <=[128,768] (2 banks/slot);
        # peak live set is {pg, pg, p14, p34} = 4 slots = all 8 banks
        pps = ctx.enter_context(tc.tile_pool(name="pps", bufs=4, space="PSUM"))

        for rep in range(repeat):
            for gi, grp in enumerate(GROUPS):
                G = len(grp)
                hts, abts = [], []
                dcolss = []
                for j, s in enumerate(grp):
                    # ---- loads (one DMA per tensor per sequence) -------
                    ht = hp.tile([P, CHUNKS * H], bf16, name=f"h{rep}_{s}", tag="h")
                    nc.sync.dma_start(ht[:], hid[s])
                    hts.append(ht)
                    abt = abp.tile(
                        [P, CHUNKS * 2 * T], bf16, name=f"a{rep}_{s}", tag="a"
                    )
                    nc.sync.dma_start(abt[:], ab[s])
                    abts.append(abt)
                    dct = dcp.tile(
                        [P, 2 * T + 4 * (CHUNKS - 1)], bf16, name=f"d{rep}_{s}", tag="d"
                    )
                    nc.sync.dma_start(dct[:], dc[s])
                    offs = [0] + [2 * T + 4 * (c - 1) for c in range(1, CHUNKS)]
                    dcols = [
                        dct[:, offs[ci] : offs[ci] + (2 * T if ci == 0 else 4)]
                        for ci in range(CHUNKS)
                    ]
                    dcolss.append(dcols)

                # group ABX block: rows 32j..32j+32 belong to sequence j,
                # so stage-C lhsT/rhs share their base partition
                axg = axp.tile([3 * 2 * T, LC], bf16, name=f"ax{rep}_{gi}", tag="ax")
                nc.sync.dma_start(
                    axg[0 : 32 * G, :],
                    abx[grp[0] : grp[0] + G].rearrange("g t l -> (g t) l"),
                )

                def hc(j, ci, n0=0, n1=H):
                    return hts[j][:, ci * H + n0 : ci * H + n1]

                # ---- stage A (whole group -> one psum tile) ------------
                p14 = pps.tile([P, H], f32, name=f"p14_{rep}_{gi}", tag="p")
                for j in range(G):
                    for ci in range(CHUNKS):
                        for n0, n1 in N_SPLITS:
                            nc.tensor.matmul(
                                p14[32 * j : 32 * j + 32, n0:n1],
                                abts[j][:, ci * 2 * T : (ci + 1) * 2 * T],
                                hc(j, ci, n0, n1),
                                start=(ci == 0),
                                stop=(ci == CHUNKS - 1),
                            )
                sb14 = s1p.tile([3 * 2 * T, H], bf16, name=f"sb14_{rep}_{gi}", tag="s")
                nc.scalar.copy(sb14[0 : 32 * G, :], p14[0 : 32 * G, :])

                # ---- stage C ladder: g = ABX @ [Q_T; R_T]; gam = h . g -
                for j in range(G):
                    for ci in range(CHUNKS):
                        pg = pps.tile([P, H], f32, name=f"pg{rep}_{grp[j]}_{ci}", tag="p")
                        for n0, n1 in N_SPLITS:
                            nc.tensor.matmul(
                                pg[:, n0:n1],
                                axg[32 * j : 32 * j + 32, ci * P : (ci + 1) * P],
                                sb14[32 * j : 32 * j + 32, n0:n1],
                                start=True,
                                stop=True,
                            )
                        scr = scp.tile([P, H], bf16, name=f"sc{rep}_{grp[j]}_{ci}", tag="sc")
                        gam = gmp.tile([P, 1], f32, name=f"g{rep}_{grp[j]}_{ci}", tag="g")
                        # gam = sum_h pg * h: DVE multiplies, ACT row-sums
                        nc.vector.tensor_mul(scr[:], pg[:], hc(j, ci))
                        scr2 = sc2p.tile([P, H], bf16, name=f"s2{rep}_{grp[j]}_{ci}", tag="s2")
                        nc.scalar.activation(scr2[:], scr[:], copy_fn, accum_out=gam[:])
                        nc.vector.tensor_scalar_mul(
                            dcolss[j][ci][:, 2:4], dcolss[j][ci][:, 0:2], gam[:]
                        )

                # ---- stage D (whole group -> one psum tile) ------------
                p34 = pps.tile([P, H], f32, name=f"p34_{rep}_{gi}", tag="p")
                for j in range(G):
                    for ci in range(CHUNKS):
                        rows = 2 * T if ci == 0 else 4
                        for n0, n1 in N_SPLITS:
                            nc.tensor.matmul(
                                p34[32 * j : 32 * j + rows, n0:n1],
                                dcolss[j][ci],
                                hc(j, ci, n0, n1),
                                start=(ci == 0),
                                stop=(ci == CHUNKS - 1),
                                skip_group_check=True,
                            )
                # one full-width copy (rows 4..31 of each strip are zeros);
                # per-strip DMAs gather the 4 live rows of each 32-row strip
                osb = osp.tile([3 * 2 * T, H], f32, name=f"o{rep}_{gi}", tag="o")
                nc.scalar.copy(osb[0 : 32 * G, :], p34[0 : 32 * G, :])
                for j, s in enumerate(grp):
                    nc.sync.dma_start(
                        out[4 * s : 4 * s + 4, :], osb[32 * j : 32 * j + 4, :]
                    )

    nc.compile()
    return nc


def _prep_core_inputs(hidden_states, attention_mask, role_ids, turn_ids):
    """Per-core input maps: one-hot / band-smeared mask prep (index work only)."""
    import ml_dtypes

    bf16 = ml_dtypes.bfloat16

    active = attention_mask != 0
    counts = active.sum(-1)
    assert counts.max() <= LC, f"active tokens {counts.max()} exceed LC={LC}"
    # stable-sort active tokens to the front, keep the first LC positions.
    # Padded positions carry real h values but zero masks, so every
    # contribution they could make is exactly zero.
    sel = np.argsort(~active, axis=1, kind="stable")[:, :LC]  # [B, LC]

    am = np.take_along_axis(active, sel, axis=1).astype(np.float32)
    ro = np.take_along_axis(role_ids, sel, axis=1)
    tu = np.take_along_axis(turn_ids, sel, axis=1)
    hidden_states = np.take_along_axis(hidden_states, sel[..., None], axis=1)

    a = am * (ro == 0)
    b = am * (ro == 1)
    onehot = (tu[..., None] == np.arange(T, dtype=tu.dtype)).astype(
        np.float32
    )  # [B, LC, T]
    A1 = onehot * a[..., None]
    B1 = onehot * b[..., None]
    band = (
        np.abs(np.arange(T)[:, None] - np.arange(T)[None, :]) <= VIEW_RANGE
    ).astype(np.float32)
    A1b = A1 @ band  # a_l * band[turn_l, :]
    B1b = B1 @ band

    # pre-staged stage-D weight template: per chunk [a, b, 0, 0] columns,
    # chunk 0 zero-padded to 32 columns
    ab2 = np.stack([a, b], axis=-1).reshape(B_SEQ, CHUNKS, P, 2)
    dc_all = np.zeros((B_SEQ, P, 2 * T + 4 * (CHUNKS - 1)), np.float32)
    dc_all[:, :, 0:2] = ab2[:, 0]
    for c in range(1, CHUNKS):
        o = 2 * T + 4 * (c - 1)
        dc_all[:, :, o : o + 2] = ab2[:, c]

    def chunked(x):
        # [10, L, F] -> [10, CHUNKS, P, F] -> [10, P, CHUNKS*F]
        f = x.shape[-1]
        return (
            x.reshape(SPC, CHUNKS, P, f)
            .transpose(0, 2, 1, 3)
            .reshape(SPC, P, CHUNKS * f)
        )

    in_maps = []
    for c in range(N_CORES):
        sl = slice(c * SPC, (c + 1) * SPC)
        in_maps.append(
            {
                "hid": np.ascontiguousarray(chunked(hidden_states[sl])).astype(bf16),
                "ab": np.ascontiguousarray(
                    chunked(np.concatenate([A1[sl], B1[sl]], axis=-1))
                ).astype(bf16),
                "dc": np.ascontiguousarray(dc_all[sl]).astype(bf16),
                "abx": np.ascontiguousarray(
                    np.concatenate([B1b[sl], A1b[sl]], axis=-1).transpose(0, 2, 1)
                ).astype(bf16),
            }
        )
    # cheap reference for a device-integrity check: qs/rs rows only (~2% of
    # the device FLOPs, recomputed on host in fp32)
    hb = hidden_states.astype(bf16).astype(np.float32)
    qs_ref = np.einsum("bl,blh->bh", a, hb)
    rs_ref = np.einsum("bl,blh->bh", b, hb)
    return in_maps, a.sum(-1), b.sum(-1), qs_ref, rs_ref


def _outputs_ok(outs, qs_ref, rs_ref):
    """Detect corrupted device runs: finite outputs + stage-A rows match host."""
    vecs = np.concatenate(outs, axis=0).reshape(-1, 4, H)
    if not np.isfinite(vecs).all():
        return False
    for got, ref in ((vecs[:, 0], qs_ref), (vecs[:, 1], rs_ref)):
        num = np.linalg.norm(got - ref, axis=-1)
        den = np.linalg.norm(ref, axis=-1) + 1e-6
        if (num / den).max() > 0.05:
            return False
    return True


def _finalize(outs, labels, na, nb):
    """Host-side O(B*H) reduction: cosine, log-softmax, label-weighted loss."""
    vecs = np.concatenate(outs, axis=0).astype(np.float64).reshape(-1, 4, H)
    qs = vecs[:, 0] / (na + AVG_EPS)[:, None]
    rs = vecs[:, 1] / (nb + AVG_EPS)[:, None]
    qc = vecs[:, 2] / (nb + AVG_EPS)[:, None]
    rc = vecs[:, 3] / (na + AVG_EPS)[:, None]

    def cos(x, y):
        nx = np.maximum(np.linalg.norm(x, axis=-1), COS_EPS)
        ny = np.maximum(np.linalg.norm(y, axis=-1), COS_EPS)
        return (x * y).sum(-1) / (nx * ny)

    logit_q = (cos(qs, qc) / TEMP).reshape(-1, SAMPLES)
    logit_r = (cos(rs, rc) / TEMP).reshape(-1, SAMPLES)

    def lsm(x):
        m = x.max(-1, keepdims=True)
        e = np.exp(x - m)
        return x - m - np.log(e.sum(-1, keepdims=True))

    lab = labels.astype(np.float64)
    loss_q = -np.mean(lsm(logit_q) * lab)
    loss_r = -np.mean(lsm(logit_r) * lab)
    return np.float32(loss_r + loss_q)


def kernel(hidden_states, labels, attention_mask, role_ids, turn_ids):
    import time

    from concourse.bass_utils import run_bass_kernel_spmd

    if "nc" not in _CACHE:
        _CACHE["nc"] = _build_nc()
    nc = _CACHE["nc"]

    in_maps, na, nb, qs_ref, rs_ref = _prep_core_inputs(
        np.asarray(hidden_states),
        np.asarray(attention_mask),
        np.asarray(role_ids),
        np.asarray(turn_ids),
    )
    trace = bool(os.environ.get("BASS_KERNEL_TRACE"))

    # the axon/NRT path very occasionally drops a run (device-unrecoverable
    # or corrupted output); validate cheaply and retry rather than fail
    outs = None
    for attempt in range(3):
        try:
            res = run_bass_kernel_spmd(
                nc, in_maps, core_ids=list(range(N_CORES)), trace=trace
            )
            cand = [res.results[c]["out"] for c in range(N_CORES)]
        except Exception:
            if attempt == 2:
                raise
            time.sleep(2.0)
            continue
        outs = cand
        if _outputs_ok(cand, qs_ref, rs_ref):
            break
    if trace:
        _CACHE["last_results"] = res
        print(
            f"[kernel] exec_time_ns={res.exec_time_ns} "
            f"mean_exec_time_ns={res.mean_exec_time_ns}"
        )
    return _finalize(outs, np.asarray(labels), na, nb)



# revision 9
# speedup vs baseline: 1.2541x; 1.2541x over previous
"""Trainium2 Bass kernel for the Dial2vec contrastive loss (nn_Dial2vec).

Math: the dense reference collapses (see v1 notes) to, per sequence,
    Q_T[t] = sum_{turn_l=t} a_l h_l ; R_T[t] likewise with b        [16, H]
    gam_l  = a_l h_l.(Band R_T)[turn_l] + b_l h_l.(Band Q_T)[turn_l]
    qs = sum a_l h_l ; qc = sum a_l gam_l h_l ; rs/rc with b
followed by a host-side O(B*H) cosine / log-softmax reduction.  Cosine
similarity is scale-invariant, so mask-count denominators and a global
2^-6 scale on gam cancel.

Device dataflow (v2) — everything runs as small PE matmuls in fp8e4
(end-to-end quantization error ~2e-4, vs the 2e-2 gate):

  G-pass   Gt[hb] = h_blk(ci,hb)^T @ ABX_ci        [128h, 32] x6, psum-acc
           (ABX = band-smeared one-hot masks, so Gt = [Band R_T; Band Q_T]^T
           directly — no separate band smear and no transposes)
  U-pass   U_ci  = hT_blk(hb,ci)^T @ Gt[hb]        [128tok, 32], psum-acc
  gam      one fused DVE tensor_tensor_reduce per chunk:
           gam = rowsum(U ∘ AB2) * 2^-6            [128, 1]
  D-pass   out = [a, b, a*gam, b*gam]^T @ h_chunk  [4, H] rows =
           [qs, rs, qc*2^-6, rc*2^-6], two sequences column-tiled per wave.

h is shipped twice (row-major for G/D, transposed for U) because the PE
contracts over partitions only; fp8 keeps the total input at ~6.3 MB/core.
Host does index-only mask prep + the final 40-vector reduction.
"""

import os

import numpy as np

B_SEQ = 80
L = 512
H = 768
SAMPLES = 10
T = 16
VIEW_RANGE = 2
TEMP = 0.2
AVG_EPS = 1e-6
COS_EPS = 1e-8

N_CORES = 8
SPC = SAMPLES  # sequences per core = one dialogue
P = 128
LC = 384  # compacted token count (attention_mask=1 tokens only, zero-padded)
CH = LC // P  # 3 chunks
HB = H // P  # 6 h-blocks
TT = 2 * T  # 32
GSC = 2.0**-6  # keeps a*gam within fp8e4 range; cancels in cosine

# merged per-sequence input row layout (fp8 bytes)
OFF_HX = 0
OFF_HXT = CH * H  # 2304
OFF_ABX = OFF_HXT + HB * LC  # 4608
OFF_AB2 = OFF_ABX + CH * TT  # 4704
OFF_ABD = OFF_AB2 + CH * TT  # 4800
MROW = OFF_ABD + CH * 4  # 4812

_CACHE: dict = {}


def _build_nc():
    from contextlib import ExitStack

    import concourse.bacc as bacc
    import concourse.mybir as mybir
    import concourse.tile as tile

    f32 = mybir.dt.float32
    bf16 = mybir.dt.bfloat16
    f8 = mybir.dt.float8e4
    add = mybir.AluOpType.add

    nc = bacc.Bacc(
        "TRN2",
        debug=False,
        enable_asserts=False,
        target_bir_lowering=False,
    )

    mg = nc.dram_tensor("mg", [SPC, P, MROW], f8, kind="ExternalInput").ap()
    out = nc.dram_tensor("out", [SPC, 4, H], f32, kind="ExternalOutput").ap()

    with tile.TileContext(nc) as tc, ExitStack() as ctx:
        mgp = ctx.enter_context(tc.tile_pool(name="mgp", bufs=4))
        gtp = ctx.enter_context(tc.tile_pool(name="gtp", bufs=3))
        scp = ctx.enter_context(tc.tile_pool(name="scp", bufs=2))
        gmp = ctx.enter_context(tc.tile_pool(name="gmp", bufs=2))
        osp = ctx.enter_context(tc.tile_pool(name="osp", bufs=2))
        pgp = ctx.enter_context(tc.tile_pool(name="pgp", bufs=2, space="PSUM"))
        pup = ctx.enter_context(tc.tile_pool(name="pup", bufs=2, space="PSUM"))
        pdp = ctx.enter_context(tc.tile_pool(name="pdp", bufs=2, space="PSUM"))

        mgs, pus, gams = {}, {}, {}

        def g_pass(s):
            mgt = mgp.tile([P, MROW], f8, name=f"mg{s}", tag="mg")
            nc.sync.dma_start(mgt[:], mg[s])
            mgs[s] = mgt
            pg = pgp.tile([P, HB * TT], f32, name=f"pg{s}", tag="pg")
            for hb in range(HB):
                for ci in range(CH):
                    nc.tensor.matmul(
                        pg[:, hb * TT : (hb + 1) * TT],
                        mgt[:, ci * H + hb * P : ci * H + (hb + 1) * P],
                        mgt[:, OFF_ABX + ci * TT : OFF_ABX + (ci + 1) * TT],
                        start=(ci == 0),
                        stop=(ci == CH - 1),
                    )
            gt = gtp.tile([P, HB * TT], f8, name=f"gt{s}", tag="gt")
            nc.scalar.copy(gt[:], pg[:])
            return gt

        def u_pass(s, gt):
            mgt = mgs[s]
            pu = pup.tile([P, CH * TT], f32, name=f"pu{s}", tag="pu")
            for ci in range(CH):
                for hb in range(HB):
                    nc.tensor.matmul(
                        pu[:, ci * TT : (ci + 1) * TT],
                        mgt[:, OFF_HXT + hb * LC + ci * P : OFF_HXT + hb * LC + (ci + 1) * P],
                        gt[:, hb * TT : (hb + 1) * TT],
                        start=(hb == 0),
                        stop=(hb == HB - 1),
                    )
            # gam = rowsum(U ∘ AB2) per chunk; tensor_tensor_reduce is broken
            # on this HW, so one DVE mul + per-chunk DVE reduces instead.
            # GSC is pre-baked into the host-side [a, b] columns of abd.
            scr = scp.tile([P, CH * TT], bf16, name=f"sc{s}", tag="sc")
            gam = gmp.tile([P, CH], f32, name=f"ga{s}", tag="ga")
            nc.vector.tensor_mul(scr[:], pu[:], mgt[:, OFF_AB2 : OFF_AB2 + CH * TT])
            for ci in range(CH):
                nc.vector.tensor_reduce(
                    gam[:, ci : ci + 1],
                    scr[:, ci * TT : (ci + 1) * TT],
                    axis=mybir.AxisListType.X,
                    op=add,
                )
                # abd chunk cols [a*GSC, b*GSC, ., .] -> cols 2:4 = gam-scaled
                nc.gpsimd.tensor_scalar_mul(
                    mgt[:, OFF_ABD + 4 * ci + 2 : OFF_ABD + 4 * ci + 4],
                    mgt[:, OFF_ABD + 4 * ci : OFF_ABD + 4 * ci + 2],
                    gam[:, ci : ci + 1],
                )

        def d_pack(s0, s1, pi):
            pd = pdp.tile([P, H], f32, name=f"pd{pi}", tag="pd")
            for j, s in ((0, s0), (1, s1)):
                mgt = mgs[s]
                for ci in range(CH):
                    for n0, n1 in ((0, 512), (512, H)):
                        nc.tensor.matmul(
                            pd[32 * j : 32 * j + 4, n0:n1],
                            mgt[:, OFF_ABD + 4 * ci : OFF_ABD + 4 * ci + 4],
                            mgt[:, ci * H + n0 : ci * H + n1],
                            start=(ci == 0),
                            stop=(ci == CH - 1),
                            skip_group_check=True,
                        )
            ot = osp.tile([36, H], f32, name=f"ot{pi}", tag="ot")
            if pi % 2 == 0:
                nc.scalar.copy(ot[0:4, :], pd[0:4, :])
                nc.scalar.copy(ot[32:36, :], pd[32:36, :])
            else:
                nc.vector.tensor_copy(ot[0:4, :], pd[0:4, :])
                nc.vector.tensor_copy(ot[32:36, :], pd[32:36, :])
            nc.sync.dma_start(out[s0], ot[0:4, :])
            nc.sync.dma_start(out[s1], ot[32:36, :])

        # software pipeline: G(s) | U(s-1) | D(s-2, s-1)
        gts = {}
        for s in range(SPC + 1):
            if s < SPC:
                gts[s] = g_pass(s)
            if s >= 1:
                u_pass(s - 1, gts.pop(s - 1))
            if s >= 2 and s % 2 == 0:
                d_pack(s - 2, s - 1, s // 2 - 1)

    nc.compile()
    return nc


def _prep_core_inputs(hidden_states, attention_mask, role_ids, turn_ids):
    """Per-core input maps: one-hot / band-smeared mask prep (index work only)."""
    import ml_dtypes

    f8 = ml_dtypes.float8_e4m3

    active = attention_mask != 0
    counts = active.sum(-1)
    assert counts.max() <= LC, f"active tokens {counts.max()} exceed LC={LC}"
    # stable-sort active tokens to the front; padded positions carry zero masks
    sel = np.argsort(~active, axis=1, kind="stable")[:, :LC]

    am = np.take_along_axis(active, sel, axis=1).astype(np.float32)
    ro = np.take_along_axis(role_ids, sel, axis=1)
    tu = np.take_along_axis(turn_ids, sel, axis=1)
    hc = np.take_along_axis(hidden_states, sel[..., None], axis=1)

    a = am * (ro == 0)
    b = am * (ro == 1)
    onehot = (tu[..., None] == np.arange(T, dtype=tu.dtype)).astype(np.float32)
    A1 = onehot * a[..., None]
    B1 = onehot * b[..., None]
    band = (
        np.abs(np.arange(T)[:, None] - np.arange(T)[None, :]) <= VIEW_RANGE
    ).astype(np.float32)
    # G = ABX^T h = [Band R_T ; Band Q_T]; gam selector AB2 = [A1 | B1]
    ABX = np.concatenate([B1 @ band, A1 @ band], axis=-1)  # [B, LC, 32]
    AB2 = np.concatenate([A1, B1], axis=-1)

    def chunked(x):
        f = x.shape[-1]
        return (
            x.reshape(B_SEQ, CH, P, f).transpose(0, 2, 1, 3).reshape(B_SEQ, P, CH * f)
        )

    hq = hc.astype(f8)  # quantize once; all views share the same values
    hx = chunked(hq.astype(np.float32)).astype(f8)  # [B, P, CH*H]
    # hxt[s, p, hb*LC + l] = h[s, l, hb*P + p]
    hxt = (
        np.ascontiguousarray(
            hq.astype(np.float32).reshape(B_SEQ, LC, HB, P).transpose(0, 3, 2, 1)
        )
        .reshape(B_SEQ, P, HB * LC)
        .astype(f8)
    )
    abx = chunked(ABX).astype(f8)
    ab2 = chunked(AB2).astype(f8)
    # [a, b] columns pre-scaled by GSC so the device-side gam products stay in
    # fp8 range; the uniform scale cancels in the cosine
    abd = np.zeros((B_SEQ, P, CH * 4), np.float32)
    ab_ch = chunked(np.stack([a, b], axis=-1)) * GSC  # [B, P, CH*2]
    for ci in range(CH):
        abd[:, :, 4 * ci : 4 * ci + 2] = ab_ch[:, :, 2 * ci : 2 * ci + 2]
    abd = abd.astype(f8)

    mgall = np.concatenate(
        [
            hx.view(np.uint8),
            hxt.view(np.uint8),
            abx.view(np.uint8),
            ab2.view(np.uint8),
            abd.view(np.uint8),
        ],
        axis=-1,
    ).view(f8)
    assert mgall.shape == (B_SEQ, P, MROW)

    in_maps = []
    for c in range(N_CORES):
        sl = slice(c * SPC, (c + 1) * SPC)
        in_maps.append({"mg": np.ascontiguousarray(mgall[sl])})

    # cheap integrity reference: qs/rs rows recomputed on host from the same fp8 h
    hq32 = hq.astype(np.float32)
    qs_ref = np.einsum("bl,blh->bh", a, hq32) * GSC
    rs_ref = np.einsum("bl,blh->bh", b, hq32) * GSC
    return in_maps, a.sum(-1), b.sum(-1), qs_ref, rs_ref


def _outputs_ok(outs, qs_ref, rs_ref):
    """Detect corrupted device runs: finite outputs + qs/rs rows match host."""
    vecs = np.concatenate(outs, axis=0).reshape(-1, 4, H)
    if not np.isfinite(vecs).all():
        return False
    for got, ref in ((vecs[:, 0], qs_ref), (vecs[:, 1], rs_ref)):
        num = np.linalg.norm(got - ref, axis=-1)
        den = np.linalg.norm(ref, axis=-1) + 1e-6
        if (num / den).max() > 0.05:
            return False
    return True


def _finalize(outs, labels, na, nb):
    """Host-side O(B*H) reduction: cosine, log-softmax, label-weighted loss.

    Rows per sequence: [qs, rs, qc*2^-6, rc*2^-6]; the scale and the
    mask-count denominators cancel inside the cosine.
    """
    vecs = np.concatenate(outs, axis=0).astype(np.float64).reshape(-1, 4, H)
    qs = vecs[:, 0] / (na + AVG_EPS)[:, None]
    rs = vecs[:, 1] / (nb + AVG_EPS)[:, None]
    qc = vecs[:, 2] / (nb + AVG_EPS)[:, None]
    rc = vecs[:, 3] / (na + AVG_EPS)[:, None]

    def cos(x, y):
        nx = np.maximum(np.linalg.norm(x, axis=-1), COS_EPS)
        ny = np.maximum(np.linalg.norm(y, axis=-1), COS_EPS)
        return (x * y).sum(-1) / (nx * ny)

    logit_q = (cos(qs, qc) / TEMP).reshape(-1, SAMPLES)
    logit_r = (cos(rs, rc) / TEMP).reshape(-1, SAMPLES)

    def lsm(x):
        m = x.max(-1, keepdims=True)
        e = np.exp(x - m)
        return x - m - np.log(e.sum(-1, keepdims=True))

    lab = labels.astype(np.float64)
    loss_q = -np.mean(lsm(logit_q) * lab)
    loss_r = -np.mean(lsm(logit_r) * lab)
    return np.float32(loss_r + loss_q)


def kernel(hidden_states, labels, attention_mask, role_ids, turn_ids):
    import time

    from concourse.bass_utils import run_bass_kernel_spmd

    if "nc" not in _CACHE:
        _CACHE["nc"] = _build_nc()
    nc = _CACHE["nc"]

    in_maps, na, nb, qs_ref, rs_ref = _prep_core_inputs(
        np.asarray(hidden_states),
        np.asarray(attention_mask),
        np.asarray(role_ids),
        np.asarray(turn_ids),
    )
    trace = bool(os.environ.get("BASS_KERNEL_TRACE"))

    # the axon/NRT path very occasionally drops a run; validate cheaply and retry
    outs = None
    for attempt in range(3):
        try:
            res = run_bass_kernel_spmd(
                nc, in_maps, core_ids=list(range(N_CORES)), trace=trace
            )
            cand = [res.results[c]["out"].reshape(SPC * 4, H) for c in range(N_CORES)]
        except Exception as e:
            import traceback

            print(f"[kernel] attempt {attempt} failed: {type(e).__name__}: {e}")
            traceback.print_exc()
            if attempt == 2:
                raise
            time.sleep(2.0)
            continue
        outs = cand
        if _outputs_ok(cand, qs_ref, rs_ref):
            break
    if trace:
        _CACHE["last_results"] = res
        print(
            f"[kernel] exec_time_ns={res.exec_time_ns} "
            f"mean_exec_time_ns={res.mean_exec_time_ns}"
        )
    return _finalize(outs, np.asarray(labels), na, nb)
